# revision 18
# baseline (speedup 1.0000x reference)
"""MultiPromptCLIP Trainium2 kernel.

Computes, for B=512, C=6, T=77, D=512, LI=193:
  global_text[b]  = text_features[b, argmax(captions[b])]
  global_image[b] = image_features[b, 0]
  local_text[b,c] = (noun_chunk_mask[b,c] @ text_features[b]) / 77
  logit_scale passthrough

Strategy: pure data-parallel over 8 NeuronCores (64 batch rows per core).
Per core, per batch row b one fp32 matmul (K=77, M=7, N=512):
  lhsT = [maskT[b]/77 | onehot(argmax capt[b])]  (77 x 7)
  rhs  = text_features[b]                        (77 x 512)
Four matmuls run concurrently in the PE array via column tiling
(tile_position=(0,32j)), writing disjoint 32-row blocks of one PSUM bank.
PSUM rows 0..5 are local_text (pre-scaled), row 6 is global_text.
argmax one-hot via enc = capt*128 + (127 - t): max picks max value with
smallest t (first occurrence, matching jnp.argmax tie-breaking); all enc
values are distinct so is_equal(enc, rowmax) is an exact one-hot.
mask/onehot transposes are done on the tensor engine with an identity.
DMA plan: big text loads alternate between the SP and ACT HWDGE rings,
grouped output stores use the opposite ring, small transfers go via SWDGE.
"""

import sys

if '/opt/trn_rl_repo' not in sys.path:
    sys.path.insert(0, '/opt/trn_rl_repo')

from contextlib import ExitStack

import numpy as np
import orjson

import concourse.bass as bass
import concourse.tile as tile
from concourse import mybir
from concourse.alu_op_type import AluOpType

N_CORES = 8
B, C, T, D, LI = 512, 6, 77, 512, 193
BL = B // N_CORES          # 64 batch rows per core
GROUP = 16                 # batch rows per text DMA group
N_GROUPS = BL // GROUP
MCH = 16                   # mask rows (b) per transpose chunk: 16*6=96 partitions
F32 = mybir.dt.float32
I32 = mybir.dt.int32
MM_DT = mybir.dt.float32   # dtype of the 64 main matmuls


# --- walrus in this container accepts only ONE sync-wait per instruction.
# Split multi-wait instructions into single-wait NoOps + the instruction.
def _split_waits_json(bir: dict) -> dict:
    for fn in bir['functions']:
        for blk in fn['blocks']:
            newinsts = []
            ctr = 0
            for ins in blk['instructions']:
                si = ins.get('sync_info')
                waits = (si or {}).get('on_wait') or []
                if len(waits) > 1:
                    for w in waits[:-1]:
                        ctr += 1
                        newinsts.append({
                            'name': f"{ins['name']}_ws{ctr}",
                            'opcode': 'NoOp',
                            'engine': ins['engine'],
                            'ins': [], 'outs': [],
                            'debug': ins.get('debug'),
                            'sync_info': {'on_update': [], 'on_wait': [w]},
                        })
                    si['on_wait'] = [waits[-1]]
                newinsts.append(ins)
            blk['instructions'] = newinsts
    return bir


def _install_wait_split_patch():
    if getattr(bass.Bass, '_wait_split_patched', False):
        return
    orig = bass.Bass.to_json_bytes

    def patched(self):
        return orjson.dumps(_split_waits_json(orjson.loads(orig(self))))

    bass.Bass.to_json_bytes = patched
    bass.Bass._wait_split_patched = True


def build_program(reps: int = 1, variant: str = "full") -> bass.Bass:
    """reps>1 repeats the whole pipeline (same inputs/outputs) for benchmarking:
    HW per-rep time = slope of total time vs reps.
    variant: 'full' | 'dma' (input DMA only) | 'nocopy' (no psum drain/store)."""
    _install_wait_split_patch()
    nc = bass.Bass("TRN2", target_bir_lowering=False, debug=False)

    tf = nc.dram_tensor("tf", (BL, T, D), F32, kind="ExternalInput")
    cap = nc.dram_tensor("cap", (BL, T), I32, kind="ExternalInput")
    msk = nc.dram_tensor("msk", (BL, C, T), I32, kind="ExternalInput")
    imgcls = nc.dram_tensor("imgcls", (BL, D), F32, kind="ExternalInput")
    ident = nc.dram_tensor("ident", (128, 128), F32, kind="ExternalInput")
    negio = nc.dram_tensor("negio", (BL, T), F32, kind="ExternalInput")

    fused = nc.dram_tensor("fused", (BL, C + 1, D), F32, kind="ExternalOutput")
    gimg = nc.dram_tensor("gimg", (BL, D), F32, kind="ExternalOutput")

    with tile.TileContext(nc) as tc, ExitStack() as ctx:
        consts = ctx.enter_context(tc.tile_pool(name="consts", bufs=1))
        wpool = ctx.enter_context(tc.tile_pool(name="wpool", bufs=2))
        prep = ctx.enter_context(tc.tile_pool(name="prep", bufs=2))
        prep_ps = ctx.enter_context(tc.tile_pool(name="prep_ps", bufs=2, space="PSUM"))
        mm_ps = ctx.enter_context(tc.tile_pool(name="mm_ps", bufs=6, space="PSUM"))
        txp = ctx.enter_context(tc.tile_pool(name="txp", bufs=3))
        outp = ctx.enter_context(tc.tile_pool(name="outp", bufs=2))

        # ---- constants
        idt = consts.tile([128, 128], F32)
        nc.gpsimd.dma_start(out=idt[:], in_=ident.ap())
        nio = consts.tile([BL, T], F32)
        nc.gpsimd.dma_start(out=nio[:], in_=negio.ap())

        for _rep in range(reps):
            # ---- EOT one-hot from captions
            ci = prep.tile([BL, T], I32, tag="prep_sb")
            nc.gpsimd.dma_start(out=ci[:], in_=cap.ap())
            cf = prep.tile([BL, T], F32, tag="prep_sb")
            nc.vector.tensor_copy(out=cf[:], in_=ci[:])
            enc = prep.tile([BL, T], F32, tag="prep_sb")
            nc.vector.tensor_scalar(out=enc[:], in0=cf[:], scalar1=128.0,
                                    scalar2=None, op0=AluOpType.mult)
            nc.vector.tensor_tensor(out=enc[:], in0=enc[:], in1=nio[:],
                                    op=AluOpType.add)
            em = prep.tile([BL, 1], F32, tag="prep_em")
            nc.vector.reduce_max(out=em[:], in_=enc[:], axis=mybir.AxisListType.X)
            oh = prep.tile([BL, T], F32, tag="prep_sb")
            nc.vector.tensor_scalar(out=oh[:], in0=enc[:], scalar1=em[:],
                                    scalar2=None, op0=AluOpType.is_equal)
            ohp = prep_ps.tile([T, BL], F32, tag="prep_ps")
            nc.tensor.transpose(ohp[:], oh[:], idt[:BL, :BL])

            # ---- combined stationary operand W (77 x 64 x 7):
            # col [b, c<6] = mask[b,c,:]/77 ; col [b, 6] = onehot row b
            W = wpool.tile([T, BL, C + 1], F32, tag="W")
            nc.vector.tensor_copy(out=W[:, :, C], in_=ohp[:])
            for k in range(BL // MCH):
                mi = prep.tile([MCH * C, T], I32, tag="prep_mi")
                nc.gpsimd.dma_start(out=mi[:], in_=msk.ap()[k * MCH:(k + 1) * MCH]
                                    .rearrange("b c t -> (b c) t"))
                mfl = prep.tile([MCH * C, T], F32, tag="prep_mf")
                nc.vector.tensor_copy(out=mfl[:], in_=mi[:])
                mtp = prep_ps.tile([T, MCH * C], F32, tag="prep_ps")
                nc.tensor.transpose(mtp[:], mfl[:], idt[:MCH * C, :MCH * C])
                nc.vector.tensor_scalar(
                    out=W[:, k * MCH:(k + 1) * MCH, 0:C],
                    in0=mtp[:].rearrange("t (b c) -> t b c", c=C),
                    scalar1=1.0 / float(T), scalar2=None, op0=AluOpType.mult)

            # ---- global_image passthrough (direct HBM->HBM copy, SWDGE)
            nc.gpsimd.dma_start(out=gimg.ap(), in_=imgcls.ap())

            # ---- main loop: per group of 8 batch rows (2 col-tiled quads)
            for g in range(N_GROUPS):
                ldeng = nc.sync if g % 2 == 0 else nc.scalar
                steng = nc.scalar if g % 2 == 0 else nc.sync
                tx = txp.tile([T, GROUP, D], F32, tag="tx")
                ldeng.dma_start(
                    out=tx[:],
                    in_=tf.ap()[g * GROUP:(g + 1) * GROUP].rearrange("b t d -> t b d"))
                if variant == "dma":
                    continue
                og = outp.tile([C + 1, GROUP, D], F32, tag="og")
                for q in range(GROUP // 4):
                    ps = mm_ps.tile([128, D], F32, tag="mm")
                    for j in range(4):
                        b = g * GROUP + q * 4 + j
                        jj = q * 4 + j
                        nc.tensor.matmul(ps[32 * j:32 * j + C + 1, :],
                                         W[:, b, :].bitcast(MM_DT),
                                         tx[:, jj, :].bitcast(MM_DT),
                                         start=True, stop=True,
                                         tile_position=(0, 32 * j))
                        if variant == "nocopy":
                            continue
                        if jj % 2 == 0:
                            nc.scalar.activation(
                                out=og[:, jj, :], in_=ps[32 * j:32 * j + C + 1, :],
                                func=mybir.ActivationFunctionType.Copy)
                        else:
                            nc.vector.tensor_copy(
                                out=og[:, jj, :], in_=ps[32 * j:32 * j + C + 1, :])
                if variant == "nocopy":
                    continue
                steng.dma_start(
                    out=fused.ap()[g * GROUP:(g + 1) * GROUP]
                    .rearrange("b c d -> c b d"),
                    in_=og[:])

    return nc


_CACHE = {}


def _get_program():
    if 'nc' not in _CACHE:
        _CACHE['nc'] = build_program()
    return _CACHE['nc']


def _make_in_maps(image_features, text_features, captions, noun_chunk_mask):
    image_features = np.asarray(image_features, dtype=np.float32)
    text_features = np.asarray(text_features, dtype=np.float32)
    captions = np.asarray(captions, dtype=np.int32)
    noun_chunk_mask = np.asarray(noun_chunk_mask, dtype=np.int32)

    imgcls = np.ascontiguousarray(image_features[:, 0, :])
    ident = np.eye(128, dtype=np.float32)
    negio = np.broadcast_to(
        (127.0 - np.arange(T, dtype=np.float32))[None, :], (BL, T)).copy()

    in_maps = []
    for i in range(N_CORES):
        s = slice(i * BL, (i + 1) * BL)
        in_maps.append({
            "tf": np.ascontiguousarray(text_features[s]),
            "cap": np.ascontiguousarray(captions[s]),
            "msk": np.ascontiguousarray(noun_chunk_mask[s]),
            "imgcls": np.ascontiguousarray(imgcls[s]),
            "ident": ident,
            "negio": negio,
        })
    return in_maps


def _assemble(results, logit_scale):
    fused = np.concatenate([results[i]["fused"] for i in range(N_CORES)], axis=0)
    gimg = np.concatenate([results[i]["gimg"] for i in range(N_CORES)], axis=0)
    global_text = np.ascontiguousarray(fused[:, C, :])
    local_text = np.ascontiguousarray(fused[:, :C, :])
    global_image = gimg
    return (global_text, global_image, local_text,
            np.asarray(logit_scale, dtype=np.float32))


def kernel(image_features, text_features, logit_scale, captions, noun_chunk_mask):
    from concourse.bass_utils import run_bass_kernel_spmd
    nc = _get_program()
    in_maps = _make_in_maps(image_features, text_features, captions, noun_chunk_mask)
    res = run_bass_kernel_spmd(nc, in_maps, core_ids=list(range(N_CORES)))
    return _assemble(res.results, logit_scale)


# revision 19
# speedup vs baseline: 1.4269x; 1.4269x over previous
"""MultiPromptCLIP Trainium2 kernel.

Computes, for B=512, C=6, T=77, D=512, LI=193:
  global_text[b]  = text_features[b, argmax(captions[b])]
  global_image[b] = image_features[b, 0]
  local_text[b,c] = (noun_chunk_mask[b,c] @ text_features[b]) / 77
  logit_scale passthrough

Strategy: pure data-parallel over 8 NeuronCores (64 batch rows per core).
Per core, per batch row b one fp32 matmul (K=77, M=7, N=512):
  lhsT = [maskT[b]/77 | onehot(argmax capt[b])]  (77 x 7)
  rhs  = text_features[b]                        (77 x 512)
Four matmuls run concurrently in the PE array via column tiling
(tile_position=(0,32j)), writing disjoint 32-row blocks of one PSUM bank.
PSUM rows 0..5 are local_text (pre-scaled), row 6 is global_text.
argmax one-hot via enc = capt*128 + (127 - t): max picks max value with
smallest t (first occurrence, matching jnp.argmax tie-breaking); all enc
values are distinct so is_equal(enc, rowmax) is an exact one-hot.
mask/onehot transposes are done on the tensor engine with an identity.
DMA plan: big text loads alternate between the SP and ACT HWDGE rings,
grouped output stores use the opposite ring, small transfers go via SWDGE.
"""

import sys

if '/opt/trn_rl_repo' not in sys.path:
    sys.path.insert(0, '/opt/trn_rl_repo')

from contextlib import ExitStack

import numpy as np
import orjson

import concourse.bass as bass
import concourse.tile as tile
from concourse import mybir
from concourse.alu_op_type import AluOpType

N_CORES = 8
B, C, T, D, LI = 512, 6, 77, 512, 193
BL = B // N_CORES          # 64 batch rows per core
GROUP = 16                 # batch rows per text DMA group
N_GROUPS = BL // GROUP
MCH = 16                   # mask rows (b) per transpose chunk: 16*6=96 partitions
F32 = mybir.dt.float32
I32 = mybir.dt.int32
MM_DT = mybir.dt.float32   # dtype of the 64 main matmuls


# --- walrus in this container accepts only ONE sync-wait per instruction.
# Split multi-wait instructions into single-wait NoOps + the instruction.
def _split_waits_json(bir: dict) -> dict:
    for fn in bir['functions']:
        for blk in fn['blocks']:
            newinsts = []
            ctr = 0
            for ins in blk['instructions']:
                si = ins.get('sync_info')
                waits = (si or {}).get('on_wait') or []
                if len(waits) > 1:
                    for w in waits[:-1]:
                        ctr += 1
                        newinsts.append({
                            'name': f"{ins['name']}_ws{ctr}",
                            'opcode': 'NoOp',
                            'engine': ins['engine'],
                            'ins': [], 'outs': [],
                            'debug': ins.get('debug'),
                            'sync_info': {'on_update': [], 'on_wait': [w]},
                        })
                    si['on_wait'] = [waits[-1]]
                newinsts.append(ins)
            blk['instructions'] = newinsts
    return bir


def _install_wait_split_patch():
    if getattr(bass.Bass, '_wait_split_patched', False):
        return
    orig = bass.Bass.to_json_bytes

    def patched(self):
        return orjson.dumps(_split_waits_json(orjson.loads(orig(self))))

    bass.Bass.to_json_bytes = patched
    bass.Bass._wait_split_patched = True


def build_program(reps: int = 1, variant: str = "full") -> bass.Bass:
    """reps>1 repeats the whole pipeline (same inputs/outputs) for benchmarking:
    HW per-rep time = slope of total time vs reps.
    variant: 'full' | 'dma' (input DMA only) | 'nocopy' (no psum drain/store)."""
    _install_wait_split_patch()
    nc = bass.Bass("TRN2", target_bir_lowering=False, debug=False)

    tf = nc.dram_tensor("tf", (BL, T, D), F32, kind="ExternalInput")
    cap = nc.dram_tensor("cap", (BL, T), I32, kind="ExternalInput")
    msk = nc.dram_tensor("msk", (BL, C, T), I32, kind="ExternalInput")
    imgcls = nc.dram_tensor("imgcls", (BL, D), F32, kind="ExternalInput")
    ident = nc.dram_tensor("ident", (128, 128), F32, kind="ExternalInput")
    negio = nc.dram_tensor("negio", (BL, T), F32, kind="ExternalInput")

    fused = nc.dram_tensor("fused", (BL, C + 1, D), F32, kind="ExternalOutput")
    gimg = nc.dram_tensor("gimg", (BL, D), F32, kind="ExternalOutput")

    with tile.TileContext(nc) as tc, ExitStack() as ctx:
        consts = ctx.enter_context(tc.tile_pool(name="consts", bufs=1))
        wpool = ctx.enter_context(tc.tile_pool(name="wpool", bufs=2))
        prep = ctx.enter_context(tc.tile_pool(name="prep", bufs=2))
        prep_ps = ctx.enter_context(tc.tile_pool(name="prep_ps", bufs=2, space="PSUM"))
        mm_ps = ctx.enter_context(tc.tile_pool(name="mm_ps", bufs=6, space="PSUM"))
        txp = ctx.enter_context(tc.tile_pool(name="txp", bufs=3))
        outp = ctx.enter_context(tc.tile_pool(name="outp", bufs=2))

        # ---- constants
        idt = consts.tile([128, 128], F32)
        nc.gpsimd.dma_start(out=idt[:], in_=ident.ap())
        nio = consts.tile([BL, T], F32)
        nc.gpsimd.dma_start(out=nio[:], in_=negio.ap())

        for _rep in range(reps):
            # ---- EOT one-hot from captions
            ci = prep.tile([BL, T], I32, tag="prep_sb")
            nc.gpsimd.dma_start(out=ci[:], in_=cap.ap())
            cf = prep.tile([BL, T], F32, tag="prep_sb")
            nc.vector.tensor_copy(out=cf[:], in_=ci[:])
            enc = prep.tile([BL, T], F32, tag="prep_sb")
            nc.vector.tensor_scalar(out=enc[:], in0=cf[:], scalar1=128.0,
                                    scalar2=None, op0=AluOpType.mult)
            nc.vector.tensor_tensor(out=enc[:], in0=enc[:], in1=nio[:],
                                    op=AluOpType.add)
            em = prep.tile([BL, 1], F32, tag="prep_em")
            nc.vector.reduce_max(out=em[:], in_=enc[:], axis=mybir.AxisListType.X)
            oh = prep.tile([BL, T], F32, tag="prep_sb")
            nc.vector.tensor_scalar(out=oh[:], in0=enc[:], scalar1=em[:],
                                    scalar2=None, op0=AluOpType.is_equal)
            ohp = prep_ps.tile([T, BL], F32, tag="prep_ps")
            nc.tensor.transpose(ohp[:], oh[:], idt[:BL, :BL])

            # ---- combined stationary operand W (77 x 64 x 7):
            # col [b, c<6] = mask[b,c,:]/77 ; col [b, 6] = onehot row b
            W = wpool.tile([T, BL, C + 1], F32, tag="W")
            nc.vector.tensor_copy(out=W[:, :, C], in_=ohp[:])
            for k in range(BL // MCH):
                mi = prep.tile([MCH * C, T], I32, tag="prep_mi")
                nc.gpsimd.dma_start(out=mi[:], in_=msk.ap()[k * MCH:(k + 1) * MCH]
                                    .rearrange("b c t -> (b c) t"))
                mfl = prep.tile([MCH * C, T], F32, tag="prep_mf")
                nc.vector.tensor_copy(out=mfl[:], in_=mi[:])
                mtp = prep_ps.tile([T, MCH * C], F32, tag="prep_ps")
                nc.tensor.transpose(mtp[:], mfl[:], idt[:MCH * C, :MCH * C])
                nc.vector.tensor_scalar(
                    out=W[:, k * MCH:(k + 1) * MCH, 0:C],
                    in0=mtp[:].rearrange("t (b c) -> t b c", c=C),
                    scalar1=1.0 / float(T), scalar2=None, op0=AluOpType.mult)

            # ---- global_image passthrough (device copy through SBUF, SWDGE)
            gi = outp.tile([128, D // 2], F32, tag="gimg")
            nc.gpsimd.dma_start(out=gi[:],
                                in_=imgcls.ap().rearrange("b (x d) -> (b x) d", x=2))
            nc.gpsimd.dma_start(out=gimg.ap().rearrange("b (x d) -> (b x) d", x=2),
                                in_=gi[:])

            # ---- main loop: per group of 8 batch rows (2 col-tiled quads)
            for g in range(N_GROUPS):
                ldeng = nc.sync if g % 2 == 0 else nc.scalar
                steng = nc.scalar if g % 2 == 0 else nc.sync
                tx = txp.tile([T, GROUP, D], F32, tag="tx")
                ldeng.dma_start(
                    out=tx[:],
                    in_=tf.ap()[g * GROUP:(g + 1) * GROUP].rearrange("b t d -> t b d"))
                if variant == "dma":
                    continue
                og = outp.tile([C + 1, GROUP, D], F32, tag="og")
                for q in range(GROUP // 4):
                    ps = mm_ps.tile([128, D], F32, tag="mm")
                    for j in range(4):
                        b = g * GROUP + q * 4 + j
                        jj = q * 4 + j
                        nc.tensor.matmul(ps[32 * j:32 * j + C + 1, :],
                                         W[:, b, :].bitcast(MM_DT),
                                         tx[:, jj, :].bitcast(MM_DT),
                                         start=True, stop=True,
                                         tile_position=(0, 32 * j))
                        if variant == "nocopy":
                            continue
                        if jj % 2 == 0:
                            nc.scalar.activation(
                                out=og[:, jj, :], in_=ps[32 * j:32 * j + C + 1, :],
                                func=mybir.ActivationFunctionType.Copy)
                        else:
                            nc.vector.tensor_copy(
                                out=og[:, jj, :], in_=ps[32 * j:32 * j + C + 1, :])
                if variant == "nocopy":
                    continue
                steng.dma_start(
                    out=fused.ap()[g * GROUP:(g + 1) * GROUP]
                    .rearrange("b c d -> c b d"),
                    in_=og[:])

    return nc


_CACHE = {}


def _get_program():
    if 'nc' not in _CACHE:
        _CACHE['nc'] = build_program()
    return _CACHE['nc']


def _make_in_maps(image_features, text_features, captions, noun_chunk_mask):
    image_features = np.asarray(image_features, dtype=np.float32)
    text_features = np.asarray(text_features, dtype=np.float32)
    captions = np.asarray(captions, dtype=np.int32)
    noun_chunk_mask = np.asarray(noun_chunk_mask, dtype=np.int32)

    imgcls = np.ascontiguousarray(image_features[:, 0, :])
    ident = np.eye(128, dtype=np.float32)
    negio = np.broadcast_to(
        (127.0 - np.arange(T, dtype=np.float32))[None, :], (BL, T)).copy()

    in_maps = []
    for i in range(N_CORES):
        s = slice(i * BL, (i + 1) * BL)
        in_maps.append({
            "tf": np.ascontiguousarray(text_features[s]),
            "cap": np.ascontiguousarray(captions[s]),
            "msk": np.ascontiguousarray(noun_chunk_mask[s]),
            "imgcls": np.ascontiguousarray(imgcls[s]),
            "ident": ident,
            "negio": negio,
        })
    return in_maps


def _assemble(results, logit_scale):
    fused = np.concatenate([results[i]["fused"] for i in range(N_CORES)], axis=0)
    gimg = np.concatenate([results[i]["gimg"] for i in range(N_CORES)], axis=0)
    global_text = np.ascontiguousarray(fused[:, C, :])
    local_text = np.ascontiguousarray(fused[:, :C, :])
    global_image = gimg
    return (global_text, global_image, local_text,
            np.asarray(logit_scale, dtype=np.float32))


def kernel(image_features, text_features, logit_scale, captions, noun_chunk_mask):
    from concourse.bass_utils import run_bass_kernel_spmd
    nc = _get_program()
    in_maps = _make_in_maps(image_features, text_features, captions, noun_chunk_mask)
    res = run_bass_kernel_spmd(nc, in_maps, core_ids=list(range(N_CORES)))
    return _assemble(res.results, logit_scale)


# revision 20
# speedup vs baseline: 1.7137x; 1.2010x over previous
"""MultiPromptCLIP Trainium2 kernel.

Computes, for B=512, C=6, T=77, D=512, LI=193:
  global_text[b]  = text_features[b, argmax(captions[b])]
  global_image[b] = image_features[b, 0]
  local_text[b,c] = (noun_chunk_mask[b,c] @ text_features[b]) / 77
  logit_scale passthrough

Strategy: pure data-parallel over 8 NeuronCores (64 batch rows per core).
Per core, per batch row b one fp32 matmul (K=77, M=7, N=512):
  lhsT = [maskT[b]/77 | onehot(argmax capt[b])]  (77 x 7)
  rhs  = text_features[b]                        (77 x 512)
Four matmuls run concurrently in the PE array via column tiling
(tile_position=(0,32j)), writing disjoint 32-row blocks of one PSUM bank.
PSUM rows 0..5 are local_text (pre-scaled), row 6 is global_text.
argmax one-hot via enc = capt*128 + (127 - t): max picks max value with
smallest t (first occurrence, matching jnp.argmax tie-breaking); all enc
values are distinct so is_equal(enc, rowmax) is an exact one-hot.
mask/onehot transposes are done on the tensor engine with an identity.
DMA plan: big text loads alternate between the SP and ACT HWDGE rings,
grouped output stores use the opposite ring, small transfers go via SWDGE.
"""

import sys

if '/opt/trn_rl_repo' not in sys.path:
    sys.path.insert(0, '/opt/trn_rl_repo')

from contextlib import ExitStack

import numpy as np
import orjson

import concourse.bass as bass
import concourse.tile as tile
from concourse import mybir
from concourse.alu_op_type import AluOpType

N_CORES = 8
B, C, T, D, LI = 512, 6, 77, 512, 193
BL = B // N_CORES          # 64 batch rows per core
GROUP = 16                 # batch rows per text DMA group
N_GROUPS = BL // GROUP
MCH = 16                   # mask rows (b) per transpose chunk: 16*6=96 partitions
F32 = mybir.dt.float32
I32 = mybir.dt.int32
MM_DT = mybir.dt.float32   # dtype of the 64 main matmuls


# --- walrus in this container accepts only ONE sync-wait per instruction.
# Split multi-wait instructions into single-wait NoOps + the instruction.
def _split_waits_json(bir: dict) -> dict:
    for fn in bir['functions']:
        for blk in fn['blocks']:
            newinsts = []
            ctr = 0
            for ins in blk['instructions']:
                si = ins.get('sync_info')
                waits = (si or {}).get('on_wait') or []
                if len(waits) > 1:
                    for w in waits[:-1]:
                        ctr += 1
                        newinsts.append({
                            'name': f"{ins['name']}_ws{ctr}",
                            'opcode': 'NoOp',
                            'engine': ins['engine'],
                            'ins': [], 'outs': [],
                            'debug': ins.get('debug'),
                            'sync_info': {'on_update': [], 'on_wait': [w]},
                        })
                    si['on_wait'] = [waits[-1]]
                newinsts.append(ins)
            blk['instructions'] = newinsts
    return bir


def _install_wait_split_patch():
    if getattr(bass.Bass, '_wait_split_patched', False):
        return
    orig = bass.Bass.to_json_bytes

    def patched(self):
        return orjson.dumps(_split_waits_json(orjson.loads(orig(self))))

    bass.Bass.to_json_bytes = patched
    bass.Bass._wait_split_patched = True


def build_program(reps: int = 1, variant: str = "full") -> bass.Bass:
    """reps>1 repeats the whole pipeline (same inputs/outputs) for benchmarking:
    HW per-rep time = slope of total time vs reps.
    variant: 'full' | 'dma' (input DMA only) | 'nocopy' (no psum drain/store)."""
    _install_wait_split_patch()
    nc = bass.Bass("TRN2", target_bir_lowering=False, debug=False)

    tf = nc.dram_tensor("tf", (BL, T, D), F32, kind="ExternalInput")
    cap = nc.dram_tensor("cap", (BL, T), I32, kind="ExternalInput")
    msk = nc.dram_tensor("msk", (BL, C, T), I32, kind="ExternalInput")
    imgcls = nc.dram_tensor("imgcls", (BL, D), F32, kind="ExternalInput")
    ident = nc.dram_tensor("ident", (128, 128), F32, kind="ExternalInput")
    negio = nc.dram_tensor("negio", (BL, T), F32, kind="ExternalInput")

    fused = nc.dram_tensor("fused", (BL, C + 1, D), F32, kind="ExternalOutput")
    gimg = nc.dram_tensor("gimg", (BL, D), F32, kind="ExternalOutput")

    with tile.TileContext(nc) as tc, ExitStack() as ctx:
        consts = ctx.enter_context(tc.tile_pool(name="consts", bufs=1))
        wpool = ctx.enter_context(tc.tile_pool(name="wpool", bufs=2))
        prep = ctx.enter_context(tc.tile_pool(name="prep", bufs=2))
        prep_ps = ctx.enter_context(tc.tile_pool(name="prep_ps", bufs=2, space="PSUM"))
        mm_ps = ctx.enter_context(tc.tile_pool(name="mm_ps", bufs=6, space="PSUM"))
        txp = ctx.enter_context(tc.tile_pool(name="txp", bufs=3))
        outp = ctx.enter_context(tc.tile_pool(name="outp", bufs=2))

        # ---- constants
        idt = consts.tile([128, 128], F32)
        nc.gpsimd.dma_start(out=idt[:], in_=ident.ap())
        nio = consts.tile([BL, T], F32)
        nc.gpsimd.dma_start(out=nio[:], in_=negio.ap())

        for _rep in range(reps):
            if variant == "dmatext":
                for g in range(N_GROUPS):
                    ldeng = nc.sync if g % 2 == 0 else nc.scalar
                    tx = txp.tile([T, GROUP, D], F32, tag="tx")
                    ldeng.dma_start(
                        out=tx[:],
                        in_=tf.ap()[g * GROUP:(g + 1) * GROUP]
                        .rearrange("b t d -> t b d"))
                continue
            # ---- EOT one-hot from captions
            ci = prep.tile([BL, T], I32, tag="prep_sb")
            nc.gpsimd.dma_start(out=ci[:], in_=cap.ap())
            cf = prep.tile([BL, T], F32, tag="prep_sb")
            nc.vector.tensor_copy(out=cf[:], in_=ci[:])
            enc = prep.tile([BL, T], F32, tag="prep_sb")
            nc.vector.tensor_scalar(out=enc[:], in0=cf[:], scalar1=128.0,
                                    scalar2=None, op0=AluOpType.mult)
            nc.vector.tensor_tensor(out=enc[:], in0=enc[:], in1=nio[:],
                                    op=AluOpType.add)
            em = prep.tile([BL, 1], F32, tag="prep_em")
            nc.vector.reduce_max(out=em[:], in_=enc[:], axis=mybir.AxisListType.X)
            oh = prep.tile([BL, T], F32, tag="prep_sb")
            nc.vector.tensor_scalar(out=oh[:], in0=enc[:], scalar1=em[:],
                                    scalar2=None, op0=AluOpType.is_equal)
            ohp = prep_ps.tile([T, BL], F32, tag="prep_ps")
            nc.tensor.transpose(ohp[:], oh[:], idt[:BL, :BL])

            # ---- combined stationary operand W (77 x 64 x 7):
            # col [b, c<6] = mask[b,c,:]/77 ; col [b, 6] = onehot row b
            W = wpool.tile([T, BL, C + 1], F32, tag="W")
            nc.vector.tensor_copy(out=W[:, :, C], in_=ohp[:])
            for k in range(BL // MCH):
                mi = prep.tile([MCH * C, T], I32, tag="prep_mi")
                nc.gpsimd.dma_start(out=mi[:], in_=msk.ap()[k * MCH:(k + 1) * MCH]
                                    .rearrange("b c t -> (b c) t"))
                mfl = prep.tile([MCH * C, T], F32, tag="prep_mf")
                nc.vector.tensor_copy(out=mfl[:], in_=mi[:])
                mtp = prep_ps.tile([T, MCH * C], F32, tag="prep_ps")
                nc.tensor.transpose(mtp[:], mfl[:], idt[:MCH * C, :MCH * C])
                nc.vector.tensor_scalar(
                    out=W[:, k * MCH:(k + 1) * MCH, 0:C],
                    in0=mtp[:].rearrange("t (b c) -> t b c", c=C),
                    scalar1=1.0 / float(T), scalar2=None, op0=AluOpType.mult)

            # ---- global_image passthrough (device copy through SBUF, SWDGE)
            gi = outp.tile([128, D // 2], F32, tag="gimg")
            nc.gpsimd.dma_start(out=gi[:],
                                in_=imgcls.ap().rearrange("b (x d) -> (b x) d", x=2))
            nc.gpsimd.dma_start(out=gimg.ap().rearrange("b (x d) -> (b x) d", x=2),
                                in_=gi[:])

            # ---- main loop: per group of 8 batch rows (2 col-tiled quads)
            for g in range(N_GROUPS):
                ldeng = nc.sync if g % 2 == 0 else nc.scalar
                steng = nc.scalar if g % 2 == 0 else nc.sync
                tx = txp.tile([T, GROUP, D], F32, tag="tx")
                ldeng.dma_start(
                    out=tx[:],
                    in_=tf.ap()[g * GROUP:(g + 1) * GROUP].rearrange("b t d -> t b d"))
                if variant == "dma":
                    continue
                og = outp.tile([C + 1, GROUP, D], F32, tag="og")
                for q in range(GROUP // 4):
                    ps = mm_ps.tile([128, D], F32, tag="mm")
                    for j in range(4):
                        b = g * GROUP + q * 4 + j
                        jj = q * 4 + j
                        nc.tensor.matmul(ps[32 * j:32 * j + C + 1, :],
                                         W[:, b, :].bitcast(MM_DT),
                                         tx[:, jj, :].bitcast(MM_DT),
                                         start=True, stop=True,
                                         tile_position=(0, 32 * j))
                        if variant == "nocopy":
                            continue
                        if jj % 2 == 0:
                            nc.scalar.activation(
                                out=og[:, jj, :], in_=ps[32 * j:32 * j + C + 1, :],
                                func=mybir.ActivationFunctionType.Copy)
                        else:
                            nc.vector.tensor_copy(
                                out=og[:, jj, :], in_=ps[32 * j:32 * j + C + 1, :])
                if variant == "nocopy":
                    continue
                steng.dma_start(
                    out=fused.ap()[g * GROUP:(g + 1) * GROUP]
                    .rearrange("b c d -> c b d"),
                    in_=og[:])

    return nc


_CACHE = {}


def _get_program():
    if 'nc' not in _CACHE:
        _CACHE['nc'] = build_program()
    return _CACHE['nc']


def _make_in_maps(image_features, text_features, captions, noun_chunk_mask):
    image_features = np.asarray(image_features, dtype=np.float32)
    text_features = np.asarray(text_features, dtype=np.float32)
    captions = np.asarray(captions, dtype=np.int32)
    noun_chunk_mask = np.asarray(noun_chunk_mask, dtype=np.int32)

    imgcls = np.ascontiguousarray(image_features[:, 0, :])
    ident = np.eye(128, dtype=np.float32)
    negio = np.broadcast_to(
        (127.0 - np.arange(T, dtype=np.float32))[None, :], (BL, T)).copy()

    in_maps = []
    for i in range(N_CORES):
        s = slice(i * BL, (i + 1) * BL)
        in_maps.append({
            "tf": np.ascontiguousarray(text_features[s]),
            "cap": np.ascontiguousarray(captions[s]),
            "msk": np.ascontiguousarray(noun_chunk_mask[s]),
            "imgcls": np.ascontiguousarray(imgcls[s]),
            "ident": ident,
            "negio": negio,
        })
    return in_maps


def _assemble(results, logit_scale):
    fused = np.concatenate([results[i]["fused"] for i in range(N_CORES)], axis=0)
    gimg = np.concatenate([results[i]["gimg"] for i in range(N_CORES)], axis=0)
    global_text = np.ascontiguousarray(fused[:, C, :])
    local_text = np.ascontiguousarray(fused[:, :C, :])
    global_image = gimg
    return (global_text, global_image, local_text,
            np.asarray(logit_scale, dtype=np.float32))


def kernel(image_features, text_features, logit_scale, captions, noun_chunk_mask):
    from concourse.bass_utils import run_bass_kernel_spmd
    nc = _get_program()
    in_maps = _make_in_maps(image_features, text_features, captions, noun_chunk_mask)
    res = run_bass_kernel_spmd(nc, in_maps, core_ids=list(range(N_CORES)))
    return _assemble(res.results, logit_scale)


# revision 21
# speedup vs baseline: 3.1626x; 1.8455x over previous
"""MultiPromptCLIP Trainium2 kernel.

Computes, for B=512, C=6, T=77, D=512, LI=193:
  global_text[b]  = text_features[b, argmax(captions[b])]
  global_image[b] = image_features[b, 0]
  local_text[b,c] = (noun_chunk_mask[b,c] @ text_features[b]) / 77
  logit_scale passthrough

Strategy: pure data-parallel over 8 NeuronCores (64 batch rows per core).
Per core, per batch row b one fp32 matmul (K=77, M=7, N=512):
  lhsT = [maskT[b]/77 | onehot(argmax capt[b])]  (77 x 7)
  rhs  = text_features[b]                        (77 x 512)
Four matmuls run concurrently in the PE array via column tiling
(tile_position=(0,32j)), writing disjoint 32-row blocks of one PSUM bank.
PSUM rows 0..5 are local_text (pre-scaled), row 6 is global_text.
argmax one-hot via enc = capt*128 + (127 - t): max picks max value with
smallest t (first occurrence, matching jnp.argmax tie-breaking); all enc
values are distinct so is_equal(enc, rowmax) is an exact one-hot.
mask/onehot transposes are done on the tensor engine with an identity.
DMA plan: big text loads alternate between the SP and ACT HWDGE rings,
grouped output stores use the opposite ring, small transfers go via SWDGE.
"""

import sys

if '/opt/trn_rl_repo' not in sys.path:
    sys.path.insert(0, '/opt/trn_rl_repo')

from contextlib import ExitStack

import numpy as np
import orjson

import concourse.bass as bass
import concourse.tile as tile
from concourse import mybir
from concourse.alu_op_type import AluOpType

N_CORES = 8
B, C, T, D, LI = 512, 6, 77, 512, 193
BL = B // N_CORES          # 64 batch rows per core
GROUP = 16                 # batch rows per text DMA group
N_GROUPS = BL // GROUP
MCH = 16                   # mask rows (b) per transpose chunk: 16*6=96 partitions
F32 = mybir.dt.float32
I32 = mybir.dt.int32
MM_DT = mybir.dt.float32   # dtype of the 64 main matmuls


# --- walrus in this container accepts only ONE sync-wait per instruction.
# Split multi-wait instructions into single-wait NoOps + the instruction.
def _split_waits_json(bir: dict) -> dict:
    for fn in bir['functions']:
        for blk in fn['blocks']:
            newinsts = []
            ctr = 0
            for ins in blk['instructions']:
                si = ins.get('sync_info')
                waits = (si or {}).get('on_wait') or []
                if len(waits) > 1:
                    for w in waits[:-1]:
                        ctr += 1
                        newinsts.append({
                            'name': f"{ins['name']}_ws{ctr}",
                            'opcode': 'NoOp',
                            'engine': ins['engine'],
                            'ins': [], 'outs': [],
                            'debug': ins.get('debug'),
                            'sync_info': {'on_update': [], 'on_wait': [w]},
                        })
                    si['on_wait'] = [waits[-1]]
                newinsts.append(ins)
            blk['instructions'] = newinsts
    return bir


def _install_wait_split_patch():
    if getattr(bass.Bass, '_wait_split_patched', False):
        return
    orig = bass.Bass.to_json_bytes

    def patched(self):
        return orjson.dumps(_split_waits_json(orjson.loads(orig(self))))

    bass.Bass.to_json_bytes = patched
    bass.Bass._wait_split_patched = True


def build_program(reps: int = 1, variant: str = "full") -> bass.Bass:
    """reps>1 repeats the whole pipeline (same inputs/outputs) for benchmarking:
    HW per-rep time = slope of total time vs reps.
    variant: 'full' | 'dma' (input DMA only) | 'nocopy' (no psum drain/store)."""
    _install_wait_split_patch()
    nc = bass.Bass("TRN2", target_bir_lowering=False, debug=False)

    tf = nc.dram_tensor("tf", (BL, T, D), F32, kind="ExternalInput")
    cap = nc.dram_tensor("cap", (BL, T), I32, kind="ExternalInput")
    msk = nc.dram_tensor("msk", (BL, C, T), I32, kind="ExternalInput")
    imgcls = nc.dram_tensor("imgcls", (BL, D), F32, kind="ExternalInput")
    ident = nc.dram_tensor("ident", (128, 128), F32, kind="ExternalInput")
    negio = nc.dram_tensor("negio", (BL, T), F32, kind="ExternalInput")

    fused = nc.dram_tensor("fused", (BL, C + 1, D), F32, kind="ExternalOutput")
    gimg = nc.dram_tensor("gimg", (BL, D), F32, kind="ExternalOutput")

    with tile.TileContext(nc) as tc, ExitStack() as ctx:
        consts = ctx.enter_context(tc.tile_pool(name="consts", bufs=1))
        wpool = ctx.enter_context(tc.tile_pool(name="wpool", bufs=2))
        prep = ctx.enter_context(tc.tile_pool(name="prep", bufs=2))
        prep_ps = ctx.enter_context(tc.tile_pool(name="prep_ps", bufs=2, space="PSUM"))
        mm_ps = ctx.enter_context(tc.tile_pool(name="mm_ps", bufs=6, space="PSUM"))
        txp = ctx.enter_context(tc.tile_pool(name="txp", bufs=3))
        outp = ctx.enter_context(tc.tile_pool(name="outp", bufs=2))

        # ---- constants
        idt = consts.tile([128, 128], F32)
        nc.gpsimd.dma_start(out=idt[:], in_=ident.ap())
        nio = consts.tile([BL, T], F32)
        nc.gpsimd.dma_start(out=nio[:], in_=negio.ap())

        for _rep in range(reps):
            if variant == "dmaflat":
                # same bytes, 128-partition flat reshape (not matmul-usable)
                for g in range(N_GROUPS):
                    ldeng = nc.sync if g % 2 == 0 else nc.scalar
                    nb = GROUP * T * D // 128
                    txf = txp.tile([128, nb], F32, tag="tx")
                    ldeng.dma_start(
                        out=txf[:],
                        in_=tf.ap()[g * GROUP:(g + 1) * GROUP]
                        .rearrange("b t d -> (b t d)")
                        .rearrange("(p n) -> p n", p=128))
                continue
            if variant == "dmatext":
                for g in range(N_GROUPS):
                    ldeng = nc.sync if g % 2 == 0 else nc.scalar
                    tx = txp.tile([T, GROUP, D], F32, tag="tx")
                    ldeng.dma_start(
                        out=tx[:],
                        in_=tf.ap()[g * GROUP:(g + 1) * GROUP]
                        .rearrange("b t d -> t b d"))
                continue
            # ---- EOT one-hot from captions
            ci = prep.tile([BL, T], I32, tag="prep_sb")
            nc.gpsimd.dma_start(out=ci[:], in_=cap.ap())
            cf = prep.tile([BL, T], F32, tag="prep_sb")
            nc.vector.tensor_copy(out=cf[:], in_=ci[:])
            enc = prep.tile([BL, T], F32, tag="prep_sb")
            nc.vector.tensor_scalar(out=enc[:], in0=cf[:], scalar1=128.0,
                                    scalar2=None, op0=AluOpType.mult)
            nc.vector.tensor_tensor(out=enc[:], in0=enc[:], in1=nio[:],
                                    op=AluOpType.add)
            em = prep.tile([BL, 1], F32, tag="prep_em")
            nc.vector.reduce_max(out=em[:], in_=enc[:], axis=mybir.AxisListType.X)
            oh = prep.tile([BL, T], F32, tag="prep_sb")
            nc.vector.tensor_scalar(out=oh[:], in0=enc[:], scalar1=em[:],
                                    scalar2=None, op0=AluOpType.is_equal)
            ohp = prep_ps.tile([T, BL], F32, tag="prep_ps")
            nc.tensor.transpose(ohp[:], oh[:], idt[:BL, :BL])

            # ---- combined stationary operand W (77 x 64 x 7):
            # col [b, c<6] = mask[b,c,:]/77 ; col [b, 6] = onehot row b
            W = wpool.tile([T, BL, C + 1], F32, tag="W")
            nc.vector.tensor_copy(out=W[:, :, C], in_=ohp[:])
            for k in range(BL // MCH):
                mi = prep.tile([MCH * C, T], I32, tag="prep_mi")
                nc.gpsimd.dma_start(out=mi[:], in_=msk.ap()[k * MCH:(k + 1) * MCH]
                                    .rearrange("b c t -> (b c) t"))
                mfl = prep.tile([MCH * C, T], F32, tag="prep_mf")
                nc.vector.tensor_copy(out=mfl[:], in_=mi[:])
                mtp = prep_ps.tile([T, MCH * C], F32, tag="prep_ps")
                nc.tensor.transpose(mtp[:], mfl[:], idt[:MCH * C, :MCH * C])
                nc.vector.tensor_scalar(
                    out=W[:, k * MCH:(k + 1) * MCH, 0:C],
                    in0=mtp[:].rearrange("t (b c) -> t b c", c=C),
                    scalar1=1.0 / float(T), scalar2=None, op0=AluOpType.mult)

            # ---- global_image passthrough (device copy through SBUF, SWDGE)
            gi = outp.tile([128, D // 2], F32, tag="gimg")
            nc.gpsimd.dma_start(out=gi[:],
                                in_=imgcls.ap().rearrange("b (x d) -> (b x) d", x=2))
            nc.gpsimd.dma_start(out=gimg.ap().rearrange("b (x d) -> (b x) d", x=2),
                                in_=gi[:])

            # ---- main loop: per group of 8 batch rows (2 col-tiled quads)
            for g in range(N_GROUPS):
                ldeng = nc.sync if g % 2 == 0 else nc.scalar
                steng = nc.scalar if g % 2 == 0 else nc.sync
                tx = txp.tile([T, GROUP, D], F32, tag="tx")
                ldeng.dma_start(
                    out=tx[:],
                    in_=tf.ap()[g * GROUP:(g + 1) * GROUP].rearrange("b t d -> t b d"))
                if variant == "dma":
                    continue
                og = outp.tile([C + 1, GROUP, D], F32, tag="og")
                for q in range(GROUP // 4):
                    ps = mm_ps.tile([128, D], F32, tag="mm")
                    for j in range(4):
                        b = g * GROUP + q * 4 + j
                        jj = q * 4 + j
                        nc.tensor.matmul(ps[32 * j:32 * j + C + 1, :],
                                         W[:, b, :].bitcast(MM_DT),
                                         tx[:, jj, :].bitcast(MM_DT),
                                         start=True, stop=True,
                                         tile_position=(0, 32 * j))
                        if variant == "nocopy":
                            continue
                        if jj % 2 == 0:
                            nc.scalar.activation(
                                out=og[:, jj, :], in_=ps[32 * j:32 * j + C + 1, :],
                                func=mybir.ActivationFunctionType.Copy)
                        else:
                            nc.vector.tensor_copy(
                                out=og[:, jj, :], in_=ps[32 * j:32 * j + C + 1, :])
                if variant == "nocopy":
                    continue
                steng.dma_start(
                    out=fused.ap()[g * GROUP:(g + 1) * GROUP]
                    .rearrange("b c d -> c b d"),
                    in_=og[:])

    return nc


_CACHE = {}


def _get_program():
    if 'nc' not in _CACHE:
        _CACHE['nc'] = build_program()
    return _CACHE['nc']


def _make_in_maps(image_features, text_features, captions, noun_chunk_mask):
    image_features = np.asarray(image_features, dtype=np.float32)
    text_features = np.asarray(text_features, dtype=np.float32)
    captions = np.asarray(captions, dtype=np.int32)
    noun_chunk_mask = np.asarray(noun_chunk_mask, dtype=np.int32)

    imgcls = np.ascontiguousarray(image_features[:, 0, :])
    ident = np.eye(128, dtype=np.float32)
    negio = np.broadcast_to(
        (127.0 - np.arange(T, dtype=np.float32))[None, :], (BL, T)).copy()

    in_maps = []
    for i in range(N_CORES):
        s = slice(i * BL, (i + 1) * BL)
        in_maps.append({
            "tf": np.ascontiguousarray(text_features[s]),
            "cap": np.ascontiguousarray(captions[s]),
            "msk": np.ascontiguousarray(noun_chunk_mask[s]),
            "imgcls": np.ascontiguousarray(imgcls[s]),
            "ident": ident,
            "negio": negio,
        })
    return in_maps


def _assemble(results, logit_scale):
    fused = np.concatenate([results[i]["fused"] for i in range(N_CORES)], axis=0)
    gimg = np.concatenate([results[i]["gimg"] for i in range(N_CORES)], axis=0)
    global_text = np.ascontiguousarray(fused[:, C, :])
    local_text = np.ascontiguousarray(fused[:, :C, :])
    global_image = gimg
    return (global_text, global_image, local_text,
            np.asarray(logit_scale, dtype=np.float32))


def kernel(image_features, text_features, logit_scale, captions, noun_chunk_mask):
    from concourse.bass_utils import run_bass_kernel_spmd
    nc = _get_program()
    in_maps = _make_in_maps(image_features, text_features, captions, noun_chunk_mask)
    res = run_bass_kernel_spmd(nc, in_maps, core_ids=list(range(N_CORES)))
    return _assemble(res.results, logit_scale)
